# revision 32
# baseline (speedup 1.0000x reference)
"""Trainium2 Bass kernel for multi-head attention (B=4, N=2048, C=512, 8 heads).

Sharding: 8 cores = (batch b = core//2) x (head-group g = core%2, 4 heads).
Per core the critical path is the ACT engine's exp stream (16.8M softmax
elements at 1 elem/lane/cycle); everything else is arranged to keep ACT
saturated:
  - scores: two heads per pass as concurrent K=64 row-tiled matmuls
    (tile_position (0,0)/(64,0)); kT/qT hold head pairs on partition halves
    with no zero padding
  - per (duo, block) scores land in one [128,1024] PSUM tile (2 banks) so
    exp is a single FD=1024 ACT instruction
  - attn@v: two heads per pass as concurrent M=64 col-tiled matmuls
    (tile_position (0,0)/(0,64)) into one [128,512] accumulator
  - softmax denominators: one 4-way col-tiled ones-pass per block into a
    single PSUM bank (32 partitions per head); per q-chunk a reciprocal +
    K=1 broadcast-matmuls + one DVE multiply per duo write normalized
    output directly into outT (partitions already aligned; no DMA shift)
  - PSUM: aux(1) + sAB(2) + sCD(2) + oAB(1) + oCD(1) + den(1) = 8 banks;
    the aux bank serves the woven phase-A/C fillers (projections, y blocks)
  - all PSUM evictions on DVE; ACT runs exp only
"""

import sys

sys.path.insert(0, "/opt/trn_rl_repo")

import numpy as np

B, N, C = 4, 2048, 512
H, D = 8, 64
SCALE = float(D) ** -0.5  # 0.125, exact in fp32
P = 128
CT = C // P  # 4 contraction tiles over channels
NT = N // P  # 16 token blocks (k)
NCORES = 8
QW = 512  # q-chunk width
QC = N // QW  # 4 q chunks

_cache = {}


def _build():
    import concourse.bacc as bacc
    import concourse.tile as tile
    from concourse import mybir

    f32 = mybir.dt.float32
    f16 = mybir.dt.float16
    u16 = mybir.dt.uint16
    EXP = mybir.ActivationFunctionType.Exp

    nc = bacc.Bacc("TRN2", target_bir_lowering=False, debug=False,
                   num_devices=NCORES)

    xT_d = nc.dram_tensor("xT", [C, N], f16, kind="ExternalInput")
    wqT_d = nc.dram_tensor("wqT", [P, CT * 256], f16, kind="ExternalInput")
    wkT_d = nc.dram_tensor("wkT", [P, CT * 256], f16, kind="ExternalInput")
    wvT_d = nc.dram_tensor("wvT", [P, CT * 256], f16, kind="ExternalInput")
    pwT_d = nc.dram_tensor("pwT", [P, 2 * C], f16, kind="ExternalInput")
    y_d = nc.dram_tensor("y", [N, C], f32, kind="ExternalOutput")

    with tile.TileContext(nc) as tc:
        with (
            tc.tile_pool(name="io", bufs=1) as io,
            tc.tile_pool(name="qk", bufs=1) as qk,
            tc.tile_pool(name="expp", bufs=8) as expp,
            tc.tile_pool(name="workp", bufs=2) as workp,
            tc.tile_pool(name="yp", bufs=3) as yp,
            tc.tile_pool(name="ps", bufs=1, space="PSUM") as ps,
        ):
            # ---- input loads. x is split into four per-chunk TILES so a
            # reader's dependency covers exactly one DMA; chunk 0 loads
            # first so the prologue projections start as early as possible.
            xT_ap = xT_d[:].rearrange("(t p) n -> p t n", p=P)
            xT_ch = [io.tile([P, CT, 512], f16, tag=f"xT{ch}",
                             name=f"xT{ch}") for ch in range(4)]
            wk_sb = io.tile([P, CT, 256], f16, tag="wk", name="wk_sb")
            wq_sb = io.tile([P, CT, 256], f16, tag="wq", name="wq_sb")
            wv_sb = io.tile([P, CT, 256], f16, tag="wv", name="wv_sb")
            pw_sb = io.tile([P, 2, C], f16, tag="pw", name="pw_sb")

            def load_x_chunk(ch):
                cs = slice(512 * ch, 512 * (ch + 1))
                nc.sync.dma_start(xT_ch[ch][:], xT_ap[:, :, cs])

            # weights ride the vector engine's DMA queue, in parallel with
            # the x chunks on the sync queue, so the prologue's inputs
            # (x chunk 0 + wk + wq) all land ~2us after the preamble
            load_x_chunk(0)
            nc.scalar.dma_start(
                wk_sb[:], wkT_d[:].rearrange("p (t m) -> p t m", t=CT))
            nc.scalar.dma_start(
                wq_sb[:], wqT_d[:].rearrange("p (t m) -> p t m", t=CT))
            nc.scalar.dma_start(
                wv_sb[:], wvT_d[:].rearrange("p (t m) -> p t m", t=CT))
            load_x_chunk(1)
            load_x_chunk(2)
            load_x_chunk(3)
            nc.scalar.dma_start(
                pw_sb[:], pwT_d[:].rearrange("p (t m) -> p t m", t=2))

            # ---- SBUF persistents ----
            qT = []
            kT = []
            vv = []
            outT = []
            for p in range(2):
                qT.append(qk.tile([P, N], f16, tag=f"qT{p}", name=f"qT{p}"))
                kT.append(qk.tile([P, N], f16, tag=f"kT{p}", name=f"kT{p}"))
                vv.append(qk.tile([P, NT, P], f16, tag=f"v{p}", name=f"v{p}"))
                outT.append(qk.tile([P, N], f16, tag=f"outT{p}",
                                    name=f"outT{p}"))
            ones_sb = io.tile([P, 64], f16, tag="ones", name="ones_sb")
            nc.vector.memset(ones_sb[:].bitcast(u16), 0x3C00)

            # trigger the ACT exp table load during the DMA ramp
            scratch1 = io.tile([1, 2], f32, tag="scratch1", name="scratch1")
            nc.vector.memset(scratch1[:], 0.0)
            nc.scalar.activation(scratch1[0:1, 0:1], scratch1[0:1, 1:2], EXP)

            # ---- PSUM helpers (bank budget: aux 1 + sAB 2 + sCD 2 + oAB 1
            #                    + oCD 1 + den 1 = 8) ----
            def aux_tile(nm):
                return ps.tile([P, 512], f32, tag="aux", name=nm)

            # ---- phase A emitters (run as prologue + fillers) ----
            # tag picks which PSUM bank the transient matmul lands in;
            # o0/o1/den banks are free until their phase-B first use, so the
            # prologue (and tail) can run 4 emitters in parallel.
            def emit_qk_chunk(p, w_sb, dst, ch, tag="aux"):
                cs = slice(512 * ch, 512 * (ch + 1))
                pc = slice(128 * p, 128 * (p + 1))
                t_ps = ps.tile([P, 512], f32, tag=tag,
                               name=f"qkps_{p}_{ch}_{w_sb.tensor.name}")
                for t in range(CT):
                    nc.tensor.matmul(
                        t_ps[:, :512],
                        lhsT=w_sb[:, t, pc],
                        rhs=xT_ch[ch][:, t, :],
                        start=(t == 0), stop=(t == CT - 1))
                nc.vector.tensor_copy(dst[:, cs], t_ps[:, :512])

            def emit_v_tile(tt, tag="aux"):
                ch, co = tt // 4, 128 * (tt % 4)
                t_ps = ps.tile([P, 512], f32, tag=tag, name=f"vps_{tt}")
                for t in range(CT):
                    nc.tensor.matmul(
                        t_ps[:, 0:256],
                        lhsT=xT_ch[ch][:, t, co:co + 128],
                        rhs=wv_sb[:, t, 0:256],
                        start=(t == 0), stop=(t == CT - 1))
                for p in range(2):
                    nc.vector.tensor_copy(vv[p][:, tt, :],
                                          t_ps[:, 128 * p:128 * (p + 1)])

            def emit_y_block(tt, tag="aux", act_evict=False, vec_dma=False):
                t_ps = ps.tile([P, 512], f32, tag=tag, name=f"yps_{tt}")
                for p in range(2):
                    nc.tensor.matmul(
                        t_ps[:, :512],
                        lhsT=outT[p][:, 128 * tt:128 * (tt + 1)],
                        rhs=pw_sb[:, p, :], start=(p == 0), stop=(p == 1))
                ys = yp.tile([P, C], f32, tag="y", name=f"ys_{tt}")
                if act_evict:
                    nc.scalar.copy(ys[:], t_ps[:, :512])
                else:
                    nc.vector.tensor_copy(ys[:], t_ps[:, :512])
                dq = nc.scalar if vec_dma else nc.sync
                dq.dma_start(y_d[128 * tt:128 * (tt + 1), :], ys[:])

            fillers = []

            def pop_fillers(k):
                for _ in range(k):
                    if fillers:
                        fillers.pop(0)()

            # ---- phase B ----
            # global block index m = qc*16 + i ; duo d in {0 (AB), 1 (CD)}
            def s_tile(d, m):
                return ps.tile([P, 1024], f32, tag=f"s{d}",
                               name=f"s{d}_{m}")

            def o_tile(d, qc):
                return ps.tile([P, 512], f32, tag=f"o{d}", name=f"o{d}_{qc}")

            def den_tile(qc):
                return ps.tile([P, 512], f32, tag="den", name=f"den_{qc}")

            s_cur = {}
            o_cur = {}
            den_cur = {}
            e_cur = {}

            def emit_scores_duo(d, m):
                qc, i = m // NT, m % NT
                qs = slice(QW * qc, QW * (qc + 1))
                ks = slice(128 * i, 128 * (i + 1))
                s = s_tile(d, m)
                s_cur[d] = s
                # two concurrent row-tiled K=64 matmuls (heads 2d, 2d+1)
                nc.tensor.matmul(
                    s[:, 0:512], lhsT=kT[d][0:64, ks],
                    rhs=qT[d][0:64, qs], start=True, stop=True)
                nc.tensor.matmul(
                    s[:, 512:1024], lhsT=kT[d][64:128, ks],
                    rhs=qT[d][64:128, qs], start=True, stop=True)

            def emit_scores(m):
                emit_scores_duo(0, m)
                emit_scores_duo(1, m)

            def emit_exp(m):
                for d in range(2):
                    e = expp.tile([P, 1024], f16, tag=f"e{d}",
                                  name=f"e{d}_{m}")
                    nc.scalar.activation(e[:], s_cur[d][:], EXP)
                    e_cur[(d, m)] = e

            # per-block denominator work is split three ways to keep every
            # engine under the ACT exp-stream floor: most blocks use the
            # PE ones-quad; GP_DEN blocks accumulate on GPSIMD (otherwise
            # idle), DVE_DEN blocks on the vector engine. Separate fp16
            # accumulators fold into den via per-q-chunk finish quads.
            GP_DEN = {3, 5, 9, 11}
            DVE_DEN = {7, 13}
            dacc_cur = {}

            def emit_avden(m):
                qc, i = m // NT, m % NT
                if i == 0:
                    for d in range(2):
                        o_cur[d] = o_tile(d, qc)
                    den_cur[0] = den_tile(qc)
                start, stop = (i == 0), (i == NT - 1)
                den = den_cur[0]
                for d in range(2):
                    e, o = e_cur[(d, m)], o_cur[d]
                    # two concurrent col-tiled M=64 matmuls (attn @ v)
                    nc.tensor.matmul(
                        o[0:64, :], lhsT=vv[d][:, i, 0:64], rhs=e[:, 0:512],
                        start=start, stop=stop)
                    nc.tensor.matmul(
                        o[64:128, :], lhsT=vv[d][:, i, 64:128],
                        rhs=e[:, 512:1024], start=start, stop=stop)
                if i in GP_DEN:
                    for d in range(2):
                        e = e_cur[(d, m)]
                        if i == min(GP_DEN):
                            dg = workp.tile([P, 1024], f16, tag=f"dgp{d}",
                                            name=f"dgp{d}_{qc}")
                            dacc_cur[("g", d)] = dg
                            nc.gpsimd.tensor_copy(dg[:], e[:])
                        else:
                            dg = dacc_cur[("g", d)]
                            nc.gpsimd.tensor_add(dg[:], dg[:], e[:])
                elif i in DVE_DEN:
                    for d in range(2):
                        e = e_cur[(d, m)]
                        if i == min(DVE_DEN):
                            dv = workp.tile([P, 1024], f16, tag=f"dvv{d}",
                                            name=f"dvv{d}_{qc}")
                            dacc_cur[("v", d)] = dv
                            nc.vector.tensor_copy(dv[:], e[:])
                        else:
                            dv = dacc_cur[("v", d)]
                            nc.vector.tensor_add(dv[:], dv[:], e[:])
                else:
                    for d in range(2):
                        e = e_cur[(d, m)]
                        for hh in range(2):
                            h = 2 * d + hh
                            # denominator: ones [128,32] col-tile at
                            # (0, 32h) (explicit tile_position: auto-derive
                            # rejects base partition 96); block 15 carries
                            # the accumulation-group stop
                            nc.tensor.matmul(
                                den[32 * h:32 * (h + 1), :],
                                lhsT=ones_sb[:, 0:32],
                                rhs=e[:, 512 * hh:512 * (hh + 1)],
                                start=start, stop=(i == NT - 1),
                                tile_position=(0, 32 * h))
                e_cur.pop((0, m - 2), None)
                e_cur.pop((1, m - 2), None)

            def emit_den_finish(qc):
                # fold both fp16 accumulators into den (PSUM accumulate
                # commutes; runs at i==15 so it's off the boundary path)
                den = den_cur[0]
                for key in (("v",), ("g",)):
                    for d in range(2):
                        dacc = dacc_cur[(key[0], d)]
                        for hh in range(2):
                            h = 2 * d + hh
                            nc.tensor.matmul(
                                den[32 * h:32 * (h + 1), :],
                                lhsT=ones_sb[:, 0:32],
                                rhs=dacc[:, 512 * hh:512 * (hh + 1)],
                                start=False, stop=False,
                                tile_position=(0, 32 * h))

            norm_st = {}

            def emit_norm_a(qc):
                # DVE-only: reciprocal + f16 cast of the denominators
                den = den_cur[0]
                rden = workp.tile([P, 512], f32, tag="rden",
                                  name=f"rden_{qc}")
                nc.vector.reciprocal_approx_fast(rden[:], den[:])
                rden16 = workp.tile([P, 512], f16, tag="rden16",
                                    name=f"rden16_{qc}")
                nc.vector.tensor_copy(rden16[:], rden[:])
                norm_st["rden16"] = rden16
                norm_st["o"] = (o_cur[0], o_cur[1])

            def emit_norm_b(qc):
                for d in range(2):
                    emit_norm_b_duo(qc, d)

            def emit_norm_b_duo(qc, d):
                # rb matmuls + eviction + normalization multiplies; emitted
                # a block later so the PE never head-of-line blocks on the
                # DVE recip chain
                qs = slice(QW * qc, QW * (qc + 1))
                rden16 = norm_st["rden16"]
                if True:
                    # broadcast 1/den rows across partition halves via K=1
                    # matmuls, then one DVE multiply into outT
                    rb = aux_tile(f"rb_{d}_{qc}")
                    for hh in range(2):
                        h = 2 * d + hh
                        r = 32 * h
                        nc.tensor.matmul(
                            rb[64 * hh:64 * (hh + 1), :],
                            lhsT=ones_sb[r:r + 1, 0:64],
                            rhs=rden16[r:r + 1, :],
                            start=True, stop=True,
                            tile_position=(r, 64 * hh))
                    rb_sb = workp.tile([P, 512], f32, tag=f"rb_sb{d}",
                                       name=f"rb_sb{d}_{qc}")
                    nc.vector.tensor_copy(rb_sb[:], rb[:])
                    nc.vector.tensor_mul(outT[d][:, qs], norm_st["o"][d][:],
                                         rb_sb[:])

            # ---- prologue: minimum work before the exp stream starts.
            # Borrow the still-idle o0/o1/den PSUM banks so the four
            # projection chunks pipeline instead of serializing on aux.
            emit_qk_chunk(0, wk_sb, kT[0], 0, tag="aux")
            emit_qk_chunk(0, wq_sb, qT[0], 0, tag="o0")
            emit_scores_duo(0, 0)
            emit_qk_chunk(1, wk_sb, kT[1], 0, tag="o1")
            emit_qk_chunk(1, wq_sb, qT[1], 0, tag="den")
            emit_scores_duo(1, 0)
            emit_v_tile(0, tag="aux")

            # fillers in deadline order (one-behind attnv gives one block of
            # grace): v tile tt is needed by iter tt+1, k chunk ch by iter
            # 4ch-1, q chunk ch by iter 16ch-1
            def add_qk(p, w, dstl, ch):
                fillers.append(lambda: emit_qk_chunk(p, w, dstl, ch))

            fillers.append(lambda: emit_v_tile(1))
            add_qk(0, wk_sb, kT[0], 1)
            add_qk(1, wk_sb, kT[1], 1)
            fillers.append(lambda: emit_v_tile(2))
            fillers.append(lambda: emit_v_tile(3))
            add_qk(0, wk_sb, kT[0], 2)
            add_qk(1, wk_sb, kT[1], 2)
            fillers.append(lambda: emit_v_tile(4))
            fillers.append(lambda: emit_v_tile(5))
            add_qk(0, wk_sb, kT[0], 3)
            add_qk(1, wk_sb, kT[1], 3)
            fillers.append(lambda: emit_v_tile(6))
            add_qk(0, wq_sb, qT[0], 1)
            fillers.append(lambda: emit_v_tile(7))
            add_qk(1, wq_sb, qT[1], 1)
            fillers.append(lambda: emit_v_tile(8))
            fillers.append(lambda: emit_v_tile(9))
            fillers.append(lambda: emit_v_tile(10))
            add_qk(0, wq_sb, qT[0], 2)
            fillers.append(lambda: emit_v_tile(11))
            fillers.append(lambda: emit_v_tile(12))
            add_qk(1, wq_sb, qT[1], 2)
            fillers.append(lambda: emit_v_tile(13))
            fillers.append(lambda: emit_v_tile(14))
            fillers.append(lambda: emit_v_tile(15))
            add_qk(0, wq_sb, qT[0], 3)
            add_qk(1, wq_sb, qT[1], 3)

            # ---- steady state: iter m runs exp(m), attn@v for block m-1
            # (one behind: its exp finished last iter, so the PE never
            # waits on e), then scores(m+1) (whose sAB WAR on exp(m) is
            # already satisfied by the time the PE reaches it) ----
            M = QC * NT
            for m in range(M):
                qc, i = divmod(m, NT)
                emit_exp(m)
                boundary = (i == 1 and m > NT)
                if boundary:
                    # qc-1 normalization consumers (gpsimd + DVE only),
                    # before avden(m-1) = block 0 of qc reuses the o banks
                    emit_norm_b(qc - 1)
                    if m + 1 < M:
                        emit_scores(m + 1)
                    emit_avden(m - 1)
                else:
                    if m >= 1:
                        emit_avden(m - 1)
                    if i == 0 and m > 0:
                        emit_norm_a(qc - 1)
                    if i == NT - 1:
                        emit_den_finish(qc)
                    if m + 1 < M:
                        emit_scores(m + 1)
                pop_fillers(2 if m < 9 else 1)
                if boundary:
                    qc_prev = qc - 1
                    for tt in range(4 * qc_prev, 4 * (qc_prev + 1)):
                        fillers.append(lambda tt=tt: emit_y_block(tt))
            # ---- tail: last block's attnv, last q-chunk normalization +
            # y blocks spread across the now-free o0/o1/den banks ----
            emit_avden(M - 1)
            emit_norm_a(QC - 1)
            pop_fillers(len(fillers))
            # tail: per-duo normalization interleaved with the y-projection
            # matmuls (p=0 runs while duo 1's chain is still on the DVE),
            # y blocks spread over the freed o0/o1/den banks, evictions
            # split DVE/ACT and DMAs split sync/scalar queues
            emit_norm_b_duo(QC - 1, 0)
            emit_norm_b_duo(QC - 1, 1)
            tags = ["aux", "o0", "o1", "den"]
            y_ps = []
            for j, tt in enumerate(range(NT - 4, NT)):
                t_ps = ps.tile([P, 512], f32, tag=tags[j], name=f"yps_{tt}")
                nc.tensor.matmul(
                    t_ps[:, :512], lhsT=outT[0][:, 128 * tt:128 * (tt + 1)],
                    rhs=pw_sb[:, 0, :], start=True, stop=False)
                y_ps.append(t_ps)
            for j, tt in enumerate(range(NT - 4, NT)):
                nc.tensor.matmul(
                    y_ps[j][:, :512],
                    lhsT=outT[1][:, 128 * tt:128 * (tt + 1)],
                    rhs=pw_sb[:, 1, :], start=False, stop=True)
            for j, tt in enumerate(range(NT - 4, NT)):
                ys = yp.tile([P, C], f32, tag=f"yt{j}", name=f"ys_{tt}")
                if j % 2 == 1:
                    nc.scalar.copy(ys[:], y_ps[j][:, :512])
                    nc.scalar.dma_start(y_d[128 * tt:128 * (tt + 1), :],
                                        ys[:])
                else:
                    nc.vector.tensor_copy(ys[:], y_ps[j][:, :512])
                    nc.sync.dma_start(y_d[128 * tt:128 * (tt + 1), :],
                                      ys[:])

    nc.finalize()
    return nc


def _get_nc():
    if "nc" not in _cache:
        _cache["nc"] = _build()
    return _cache["nc"]


def _pack(wt, groups):
    # [G*128, M] row-major -> [128, G*M]: partition p holds the concat over
    # groups of row (g*128 + p), so the DMA reads one contiguous run per p
    g128, m = wt.shape
    assert g128 == groups * 128
    return np.ascontiguousarray(
        wt.reshape(groups, 128, m).transpose(1, 0, 2).reshape(128, groups * m))


def _make_in_maps(x, q_w, kv_w, proj_w):
    x = np.asarray(x, dtype=np.float32)
    q_w = np.asarray(q_w, dtype=np.float32)
    kv_w = np.asarray(kv_w, dtype=np.float32)
    proj_w = np.asarray(proj_w, dtype=np.float32)
    f16 = np.float16
    in_maps = []
    for core in range(NCORES):
        b, g = core // 2, core % 2
        hs = slice(g * 256, (g + 1) * 256)
        in_maps.append({
            "xT": np.ascontiguousarray(x[b].T.astype(f16)),
            "wqT": _pack((q_w[hs, :] * np.float32(SCALE)).T.astype(f16), CT),
            "wkT": _pack(kv_w[hs, :].T.astype(f16), CT),
            "wvT": _pack(
                kv_w[C + g * 256:C + (g + 1) * 256, :].T.astype(f16), CT),
            "pwT": _pack(proj_w[:, hs].T.astype(f16), 2),
        })
    return in_maps


def kernel(x, q_w, kv_w, proj_w, proj_b, H=None, W=None, _trace=False):
    from concourse.bass_utils import run_bass_kernel_spmd

    nc = _get_nc()
    in_maps = _make_in_maps(x, q_w, kv_w, proj_w)
    res = run_bass_kernel_spmd(nc, in_maps, core_ids=list(range(NCORES)),
                               trace=_trace)
    proj_b = np.asarray(proj_b, dtype=np.float32)
    out = np.empty((B, N, C), dtype=np.float32)
    for b in range(B):
        out[b] = res.results[2 * b]["y"] + res.results[2 * b + 1]["y"] + proj_b
    if _trace:
        return out, res
    return out
